# revision 7
# baseline (speedup 1.0000x reference)
"""Trainium2 Bass kernel for the int8-fake-quant double-conv model.

Math: all fake-quantized values are integers times power-of-2 scales, so every
intermediate is exactly representable in bf16 (|int| <= 256) and every conv
accumulation is exact in fp32 PSUM (|int| < 2^24). The convs are mapped onto
the 128x128 PE array with a banded-Toeplitz stationary matrix:
  K = (cin, input-row window), M = (cout, output-row block), N = image columns,
accumulating the 3 horizontal taps as 3 PSUM-accumulated matmuls (rhs shifted
along the free dim). Per-tensor bias is folded in as an extra K row against a
constant ones-row. Rounding to the quant grid uses the fp32 magic-number trick
(add/sub 1.5*2^23*scale), which is RNE and matches jnp.round exactly.

Sharding: pure data-parallel over batch (32 -> 4 per core x 8 cores).
"""

import numpy as np
import ml_dtypes

import concourse.bacc as bacc
import concourse.bass as bass
import concourse.mybir as mybir
import concourse.tile as tile
from concourse import bass_utils

BF16 = ml_dtypes.bfloat16
N_CORES = 8
B_PER_CORE = 4
H = W = 512
H1, W1 = 510, 510      # conv1 output
H2, W2 = 508, 508      # conv2 output
CIN, CMID, COUT = 5, 10, 10
BLK = 10               # z rows per block
NBLK = 51              # 50 full + 1 edge block covers 508 z rows

# main blocks: conv1 makes 12 yq rows from 14 x rows; conv2 makes 10 z rows
# edge block (m=50): conv1 makes 10 yq rows from 12 x rows; conv2 makes 8 z rows

_prog_cache = {}


def _toeplitz(wq, cin, win, outr, dj):
    """S[(ci,i'), (co,il)] = wq[co,ci,i'-il,dj] for 0<=i'-il<=2 else 0."""
    cout = wq.shape[0]
    S = np.zeros((cin * win, cout * outr), np.float32)
    for di in range(3):
        w = wq[:, :, di, dj]                      # [co, ci]
        for il in range(outr):
            ip = il + di
            if ip >= win:
                continue
            for ci in range(cin):
                S[ci * win + ip, il::outr] = w[:, ci]
    return S


def _make_consts(w1, b1, w2, b2, s_in, s_w1, s_o1, s_w2, s_o2):
    s_in, s_w1, s_o1, s_w2, s_o2 = (float(np.asarray(v).reshape(-1)[0])
                                    for v in (s_in, s_w1, s_o1, s_w2, s_o2))
    for s in (s_in, s_w1, s_o1, s_w2, s_o2):
        m, e = np.frexp(np.float64(s))
        assert m == 0.5, f"scale {s} not a power of two; exact path invalid"

    def fq(a, s):
        return (np.clip(np.rint(a.astype(np.float32) / np.float32(s)),
                        -128, 127) * np.float32(s)).astype(np.float32)

    w1q = fq(w1, s_w1)
    b1q = fq(b1, s_in * s_w1)
    w2q = fq(w2, s_w2)
    b2q = fq(b2, s_o1 * s_w2)

    def bias_row(S, bq, outr):
        return np.concatenate([S, np.repeat(bq, outr)[None, :]], 0)

    c = {}
    for dj in range(3):
        c[f"s1_{dj}"] = _toeplitz(w1q, CIN, 14, 12, dj)
        c[f"s1e_{dj}"] = _toeplitz(w1q, CIN, 12, 10, dj)
        c[f"s2_{dj}"] = _toeplitz(w2q, CMID, 12, 10, dj)
        c[f"s2e_{dj}"] = _toeplitz(w2q, CMID, 10, 8, dj)
    c["s1_0"] = bias_row(c["s1_0"], b1q, 12)      # [71,120]
    c["s1e_0"] = bias_row(c["s1e_0"], b1q, 10)    # [61,100]
    c["s2_0"] = bias_row(c["s2_0"], b2q, 10)      # [121,100]
    c["s2e_0"] = bias_row(c["s2e_0"], b2q, 8)     # [101,80]
    consts = {k: v.astype(BF16) for k, v in c.items()}
    # exactness guard: bf16 cast must be lossless
    for k, v in c.items():
        assert np.array_equal(consts[k].astype(np.float32), v), k
    scal = {"m4x": np.float32(1.5 * 2**23 * s_in),
            "m4y": np.float32(1.5 * 2**23 * s_o1),
            "m4z": np.float32(1.5 * 2**23 * s_o2),
            "zhi": np.float32(127 * s_o2), "zlo": np.float32(-128 * s_o2)}
    return consts, scal


def build_program(scal, repeat=1):
    """Build + compile the per-core Bass program. scal holds the magic/clamp
    constants (baked in as immediates)."""
    nc = bacc.Bacc("TRN2", target_bir_lowering=False, debug=False,
                   num_devices=N_CORES)
    f32, bf = mybir.dt.float32, mybir.dt.bfloat16
    ADD, SUB = mybir.AluOpType.add, mybir.AluOpType.subtract
    MIN, MAX = mybir.AluOpType.min, mybir.AluOpType.max
    COPY = mybir.ActivationFunctionType.Copy

    x_d = nc.dram_tensor("x", [B_PER_CORE, CIN, H, W], f32, kind="ExternalInput")
    out_d = nc.dram_tensor("out", [B_PER_CORE, COUT, H2, W2], f32,
                           kind="ExternalOutput")
    s_shapes = {"s1_0": (71, 120), "s1_1": (70, 120), "s1_2": (70, 120),
                "s1e_0": (61, 100), "s1e_1": (60, 100), "s1e_2": (60, 100),
                "s2_0": (121, 100), "s2_1": (120, 100), "s2_2": (120, 100),
                "s2e_0": (101, 80), "s2e_1": (100, 80), "s2e_2": (100, 80)}
    s_d = {k: nc.dram_tensor(k, list(sh), bf, kind="ExternalInput")
           for k, sh in s_shapes.items()}
    ones_d = nc.dram_tensor("ones", [1, W], bf, kind="ExternalInput")

    m4x, m4y, m4z = (float(scal["m4x"]), float(scal["m4y"]), float(scal["m4z"]))
    zhi, zlo = float(scal["zhi"]), float(scal["zlo"])

    with tile.TileContext(nc) as tc:
        with (tc.tile_pool(name="consts", bufs=1) as cpool,
              tc.tile_pool(name="xraw", bufs=4) as xraw_pool,
              tc.tile_pool(name="xq", bufs=1) as xq_pool,
              tc.tile_pool(name="yq", bufs=1) as yq_pool,
              tc.tile_pool(name="ztmp", bufs=3) as ztmp_pool,
              tc.tile_pool(name="zout", bufs=4) as zout_pool,
              tc.tile_pool(name="p1", bufs=2, space=bass.MemorySpace.PSUM) as p1_pool,
              tc.tile_pool(name="p2", bufs=2, space=bass.MemorySpace.PSUM) as p2_pool):

            s_t = {}
            for k, sh in s_shapes.items():
                s_t[k] = cpool.tile(list(sh), bf, tag=k, name=k)
                nc.sync.dma_start(s_t[k][:], s_d[k].ap())

            # ring buffers with a persistent ones-row after the data rows
            XR, YR = 3, 3
            xq_ring = [xq_pool.tile([71, W], bf, tag=f"xq{i}", name=f"xq{i}") for i in range(XR)]
            yq_ring = [yq_pool.tile([121, W1 + 2], bf, tag=f"yq{i}", name=f"yq{i}") for i in range(YR)]
            xq_edge = xq_pool.tile([61, W], bf, tag="xqe")
            yq_edge = yq_pool.tile([101, W1 + 2], bf, tag="yqe")
            for t in xq_ring:
                nc.sync.dma_start(t[70:71, :], ones_d.ap())
            for t in yq_ring:
                nc.sync.dma_start(t[120:121, :], ones_d.ap()[:, 0:W1 + 2])
            nc.sync.dma_start(xq_edge[60:61, :], ones_d.ap())
            nc.sync.dma_start(yq_edge[100:101, :], ones_d.ap()[:, 0:W1 + 2])

            def body():
                it = 0
                for b in range(B_PER_CORE):
                    for m in range(NBLK):
                        nonlocal_ = None  # noqa
                        edge = (m == NBLK - 1)
                        r0 = BLK * m
                        xwin = 12 if edge else 14
                        yrows = 10 if edge else 12
                        zrows = 8 if edge else 10
                        kx = CIN * xwin            # 60 / 70
                        my = CMID * yrows          # 100 / 120
                        ky = CMID * yrows          # 100 / 120
                        mz = COUT * zrows          # 80 / 100
                        xq_t = xq_edge if edge else xq_ring[it % XR]
                        yq_t = yq_edge if edge else yq_ring[it % YR]
                        s1c = ("s1e_0", "s1e_1", "s1e_2") if edge else \
                              ("s1_0", "s1_1", "s1_2")
                        s2c = ("s2e_0", "s2e_1", "s2e_2") if edge else \
                              ("s2_0", "s2_1", "s2_2")

                        xr = xraw_pool.tile([70, W], f32, tag="xr", name="xr")
                        src = x_d.ap()[b, :, r0:r0 + xwin, :]
                        nc.sync.dma_start(xr[0:kx, :], src)
                        # quantize x to s_in grid (magic round), fp32 -> bf16
                        nc.vector.tensor_scalar(xq_t[0:kx, :], xr[0:kx, :],
                                                m4x, m4x, ADD, SUB)

                        p1 = p1_pool.tile([120, W1], f32, tag="p1", name="p1")
                        nc.tensor.matmul(p1[0:my, :], s_t[s1c[0]][:, 0:my],
                                         xq_t[0:kx + 1, 0:W1],
                                         start=True, stop=False)
                        nc.tensor.matmul(p1[0:my, :], s_t[s1c[1]][:, 0:my],
                                         xq_t[0:kx, 1:1 + W1],
                                         start=False, stop=False)
                        nc.tensor.matmul(p1[0:my, :], s_t[s1c[2]][:, 0:my],
                                         xq_t[0:kx, 2:2 + W1],
                                         start=False, stop=True)

                        # y: add magic, subtract magic (-> round to s_o1 grid),
                        # cast to bf16 into the yq ring tile
                        nc.scalar.activation(p1[0:my, :], p1[0:my, :], COPY,
                                             bias=m4y, scale=1.0)
                        nc.scalar.activation(yq_t[0:ky, 0:W1], p1[0:my, :], COPY,
                                             bias=-m4y, scale=1.0)

                        p2 = p2_pool.tile([100, W2], f32, tag="p2", name="p2")
                        nc.tensor.matmul(p2[0:mz, :], s_t[s2c[0]][:, 0:mz],
                                         yq_t[0:ky + 1, 0:W2],
                                         start=True, stop=False)
                        nc.tensor.matmul(p2[0:mz, :], s_t[s2c[1]][:, 0:mz],
                                         yq_t[0:ky, 1:1 + W2],
                                         start=False, stop=False)
                        nc.tensor.matmul(p2[0:mz, :], s_t[s2c[2]][:, 0:mz],
                                         yq_t[0:ky, 2:2 + W2],
                                         start=False, stop=True)

                        zt = ztmp_pool.tile([100, W2], f32, tag="zt", name="zt")
                        nc.vector.tensor_scalar(zt[0:mz, :], p2[0:mz, :],
                                                m4z, m4z, ADD, SUB)
                        zo = zout_pool.tile([100, W2], f32, tag="zo", name="zo")
                        nc.vector.tensor_scalar(zo[0:mz, :], zt[0:mz, :],
                                                zhi, zlo, MIN, MAX)
                        dst = out_d.ap()[b, :, r0:r0 + zrows, :]
                        nc.sync.dma_start(dst, zo[0:mz, :])
                        it += 1

            for _ in range(repeat):
                body()

    nc.compile()
    return nc


def _get_prog(scal_key, scal, repeat=1):
    key = (scal_key, repeat)
    if key not in _prog_cache:
        _prog_cache[key] = build_program(scal, repeat=repeat)
    return _prog_cache[key]


def kernel(x, w1, b1, w2, b2, s_in, s_w1, s_o1, s_w2, s_o2):
    x = np.ascontiguousarray(np.asarray(x, dtype=np.float32))
    assert x.shape == (32, CIN, H, W)
    consts, scal = _make_consts(np.asarray(w1), np.asarray(b1), np.asarray(w2),
                                np.asarray(b2), s_in, s_w1, s_o1, s_w2, s_o2)
    scal_key = tuple(sorted((k, float(v)) for k, v in scal.items()))
    nc = _get_prog(scal_key, scal, repeat=1)
    in_maps = []
    for c in range(N_CORES):
        m = {"x": x[c * B_PER_CORE:(c + 1) * B_PER_CORE],
             "ones": np.ones((1, W), dtype=BF16)}
        m.update(consts)
        in_maps.append(m)
    res = bass_utils.run_bass_kernel_spmd(nc, in_maps, core_ids=list(range(N_CORES)))
    return np.concatenate([res.results[c]["out"] for c in range(N_CORES)], axis=0)
